# revision 1
# baseline (speedup 1.0000x reference)
"""Trainium2 Bass kernel for nn_CE_55937654063537.

Reference computation:
    b1 = conv3x3(x, g_w) + g_b            [B, 2, 512, 512]
    b2 = conv1x1(x, theta_w) + theta_b    [B, 2, 512, 512]
    m  = patch_mean(b1, 7) + patch_mean(b2, 7)   [B, 2, 7, 7]
    out = bilinear_upsample(m, 512, 512)  (half-pixel centers)

Everything is linear, so the kernel never materializes the conv outputs.
patch_mean(feat)[i, j] is (1/(H*W)) * the sum of feat over a rectangle that is
the full map minus <=3 boundary rows/cols.  Those rectangle sums are linear in
(a) the column-sum over h of x and (b) 8 boundary rows of x.

v2 layout: host reshapes each batch's x [4, 512, 512] to [128, 16, 512]
(partition p = 16 consecutive rows of the flattened (c, h) space = 32 KB
contiguous HBM per partition).  Per batch:
  load:    two 2 MB DMAs on the sync HWDGE ring; the 8 boundary rows arrive
           via one small DMA from a host-pregathered edge tensor
  phase 1: per-channel column sums via ONE accumulating PE matmul chain
           against a [128, 4] channel-indicator lhsT (fp32r, exact for 0/1)
  phase 2: stats reduced to per-row [total + 8 edge columns] (batched DVE
           ops), then tiny fp32 matmuls -> R[co, i] summaries
  phase 3: PE transpose + small matmul against L -> m^T
  phase 4: out = A @ m @ A^T via fp32r matmuls; both channels packed into one
           [128, 2, 4, 512] SBUF tile, stored as ONE 2 MB DMA on the scalar
           HWDGE ring (loads and stores never share a ring -> no
           head-of-line blocking between them).

Data parallel over batch: 8 cores x 4 batches each; params replicated.
"""
import numpy as np

H = W = 512
K = 7
CIN = 4
CO = 2
BLOC = 4    # batches per core
NCORES = 8

_PROG = None          # cached Bass program (weight-independent; weights are inputs)
N_REPS = 1            # unroll the whole kernel body this many times (timing proxy)
XBUFS = 3             # x-tile buffering depth
LOAD_SPLIT = 2        # DMAs per 4 MB batch load
STORE_SPLIT = 2       # 1 = fused 2 MB store, 2 = per-channel 1 MB stores
TRACE = False
LAST_EXEC_NS = None
LAST_TRACE_PATH = None


# ---------------------------------------------------------------------------
# host-side constant builders (all tiny, derived from conv weights)
# ---------------------------------------------------------------------------

def resize_mat(in_size, out_size):
    """Bilinear (half-pixel, edge-normalized) interpolation matrix [out, in],
    matching jax.image.resize(method='bilinear') for upsampling."""
    inv_scale = in_size / out_size
    sample_f = (np.arange(out_size) + 0.5) * inv_scale - 0.5
    xw = np.abs(sample_f[None, :] - np.arange(in_size)[:, None])
    weights = np.maximum(0, 1 - xw)
    total = weights.sum(axis=0, keepdims=True)
    return (weights / total).T.astype(np.float32)  # [out, in]


def build_lhsTR(g_w, g_b, theta_w, theta_b):
    """Phase-2 weight blocks (per batch; identical for every b).

    Returns (blk [4, 3, 9, 14], bias [1, 14]):
      blk[ci, dw, q, col]: coefficient of stats row q of channel ci
        (q: 0=colsum over h, 1..4=x rows 0..3, 5..8=x rows 508..511)
        in output row col = co*7 + i -> R[co, i][w] under w-shift dw.
      bias[0, col]: additive constant (applies to every w of R[col]).
    """
    gw = g_w.astype(np.float64)
    gb = g_b.astype(np.float64)
    tw = theta_w.astype(np.float64)[:, :, 0, 0]
    tb = theta_b.astype(np.float64)
    blk = np.zeros((CIN, 3, 9, 14), dtype=np.float64)
    bias = np.zeros((1, 14), dtype=np.float64)

    def add_F(col, co, dw, sign):
        for ci in range(CIN):
            blk[ci, dw, 0, col] += sign * gw[co, ci, :, dw].sum()
            blk[ci, dw, 1, col] += -sign * gw[co, ci, 2, dw]   # x row 0
            blk[ci, dw, 8, col] += -sign * gw[co, ci, 0, dw]   # x row 511
            if dw == 1:
                blk[ci, dw, 0, col] += sign * tw[co, ci]
        if dw == 1:
            bias[0, col] += sign * H * (gb[co] + tb[co])

    def add_bd(col, co, r, dw, sign):
        for ci in range(CIN):
            for dh in range(3):
                hr = r + dh - 1
                if 0 <= hr < H:
                    q = 1 + hr if hr <= 3 else 5 + (hr - (H - 4))
                    blk[ci, dw, q, col] += sign * gw[co, ci, dh, dw]
            if dw == 1:
                q = 1 + r if r <= 3 else 5 + (r - (H - 4))
                blk[ci, dw, q, col] += sign * tw[co, ci]
        if dw == 1:
            bias[0, col] += sign * (gb[co] + tb[co])

    for co in range(CO):
        for i in range(K):
            col = co * 7 + i
            for dw in range(3):
                add_F(col, co, dw, 1.0)
                if i < 3:
                    for r in range(H - 3 + i, H):
                        add_bd(col, co, r, dw, -1.0)
                elif i > 3:
                    for r in range(0, i - 3):
                        add_bd(col, co, r, dw, -1.0)
    return blk.astype(np.float32), bias.astype(np.float32)


def build_L():
    """Phase-3 lhsT [7, 7] (includes the 1/(H*W) patch-mean scale)."""
    L = np.zeros((7, 7), dtype=np.float64)
    L[0, :] = 1.0
    for j in range(3):            # j=0,1,2: subtract tail elements w >= 509+j
        for e in range(3 + j, 6):
            L[1 + e, j] = -1.0    # e=3,4,5 -> rows 4..6
    for j in range(4, 7):         # j=4,5,6: subtract head elements w < j-3
        for e in range(0, j - 3):
            L[1 + e, j] = -1.0    # e=0,1,2 -> rows 1..3
    return (L / (H * W)).astype(np.float32)


def build_consts(g_w, g_b, theta_w, theta_b):
    blk, biasrow = build_lhsTR(g_w, g_b, theta_w, theta_b)
    A = resize_mat(K, H)          # [512, 7]
    biaspat = np.ones((1, 7), dtype=np.float32)
    biaspat[0, 0] = float(W)      # total-sum column gets bias once per w
    # channel-indicator lhsT; channel c's column sum lands on PSUM partition c
    ind = np.zeros((128, CIN), dtype=np.float32)
    for c in range(CIN):
        ind[32 * c:32 * (c + 1), c] = 1.0
    la = build_L().astype(np.float64) @ A.astype(np.float64).T     # [7, 512]
    # stats live on 36 partitions: q*4 + ci (q: 0=colsum, 1..8=edge rows)
    blk4 = np.ascontiguousarray(blk.transpose(2, 0, 1, 3).reshape(36, 3, 14))
    return {
        "blk": blk4,
        "biasrow": biasrow,
        "biaspat": biaspat,
        "la": np.ascontiguousarray(la.astype(np.float32)),
        "atr": np.ascontiguousarray(
            A.reshape(128, 4, K).transpose(1, 2, 0)),                 # [4, 7, 128]
        "ind": ind,
    }


# ---------------------------------------------------------------------------
# device program
# ---------------------------------------------------------------------------

def build_program():
    import concourse.bass as bass
    import concourse.bacc as bacc
    import concourse.tile as tile
    from concourse import mybir

    f32 = mybir.dt.float32
    f32r = mybir.dt.float32r
    nc = bacc.Bacc(None, target_bir_lowering=False, enable_partition_id=False)

    xs = nc.dram_tensor("xs", [BLOC, 128, 16, W], f32r, kind="ExternalInput")
    xe_d = nc.dram_tensor("xe", [BLOC, 2, 4, CIN, W], f32r, kind="ExternalInput")
    blk_d = nc.dram_tensor("blk", [36, 3, 14], f32, kind="ExternalInput")
    bias_d = nc.dram_tensor("biasrow", [1, 14], f32, kind="ExternalInput")
    bpat_d = nc.dram_tensor("biaspat", [1, 7], f32, kind="ExternalInput")
    la_d = nc.dram_tensor("la", [7, 512], f32r, kind="ExternalInput")
    atr_d = nc.dram_tensor("atr", [4, 7, 128], f32r, kind="ExternalInput")
    ind_d = nc.dram_tensor("ind", [128, CIN], f32r, kind="ExternalInput")
    y = nc.dram_tensor("y", [BLOC, CO, H, W], f32, kind="ExternalOutput")

    with tile.TileContext(nc) as tc:
        with (
            tc.tile_pool(name="consts", bufs=1) as consts,
            tc.tile_pool(name="xpool", bufs=XBUFS) as xpool,
            tc.tile_pool(name="spool", bufs=5) as spool,
            tc.tile_pool(name="vpool", bufs=2) as vpool,
            tc.tile_pool(name="etp", bufs=2) as etp,
            tc.tile_pool(name="tgpool", bufs=2) as tgpool,
            tc.tile_pool(name="obuf", bufs=2) as obuf,
            tc.tile_pool(name="pstats", bufs=2, space="PSUM") as pstats,
            tc.tile_pool(name="pr", bufs=2, space="PSUM") as pr,
            tc.tile_pool(name="ptg", bufs=2, space="PSUM") as ptg,
            tc.tile_pool(name="poc", bufs=2, space="PSUM") as poc,
        ):
            c_ind = consts.tile([128, CIN], f32r)
            nc.scalar.dma_start(out=c_ind, in_=ind_d[:, :])
            c_blk = consts.tile([36, 3, 14], f32)
            nc.scalar.dma_start(out=c_blk, in_=blk_d[:, :, :])
            c_bias = consts.tile([1, 14], f32)
            nc.scalar.dma_start(out=c_bias, in_=bias_d[:, :])
            c_bpat = consts.tile([1, 7], f32)
            nc.scalar.dma_start(out=c_bpat, in_=bpat_d[:, :])
            c_la = consts.tile([7, 512], f32r)
            nc.scalar.dma_start(out=c_la, in_=la_d[:, :])
            c_atr = consts.tile([7, 4, 128], f32r)
            nc.scalar.dma_start(out=c_atr, in_=atr_d.rearrange("t j p -> j t p"))

            def emit_once():
                # edge rows for every batch upfront on the store ring: the
                # load ring carries nothing but the eight 2 MB streams
                Ss = []
                for b in range(BLOC):
                    S = spool.tile([36, W], f32r, tag="S")
                    nc.gpsimd.dma_start(
                        out=S[4:36, :],
                        in_=xe_d[b].rearrange("e r c w -> (e r c) w"),
                    )
                    Ss.append(S)

                def load(b):
                    # ---- stream x[b]: halves split over both HWDGE rings ----
                    xt = xpool.tile([128, 16, W], f32r, tag="xt")
                    rows = 16 // LOAD_SPLIT
                    for s in range(LOAD_SPLIT):
                        eng = nc.sync if s % 2 == 0 else nc.scalar
                        eng.dma_start(
                            out=xt[:, s * rows:(s + 1) * rows, :],
                            in_=xs[b, :, s * rows:(s + 1) * rows, :])
                    return xt, Ss[b]

                def colsum(b, xt, S):
                    # ---- phase 1: per-channel column sums via indicator ----
                    st = pstats.tile([CIN, W], f32, tag="st")
                    for t in range(16):
                        nc.tensor.matmul(st, c_ind, xt[:, t, :],
                                         start=(t == 0), stop=(t == 15))
                    nc.vector.tensor_copy(S[0:CIN, :], st)

                def stage_v(b, S, ctx):
                    # ---- phase 2a: per-row summaries V = [T | edges] ----
                    # V column groups, one per w-shift dw (7 cols each):
                    #  dw=0: [T-S511, 0,  S0, S1, S508, S509, S510]
                    #  dw=1: [T,      S0, S1, S2, S509, S510, S511]
                    #  dw=2: [T-S0,   S1, S2, S3, S510, S511, 0   ]
                    V = vpool.tile([36, 21], f32, tag="V")
                    nc.vector.reduce_sum(V[:, 7:8], S, axis=mybir.AxisListType.X)
                    edges = bass.AP(           # S columns {0,1,2, 509,510,511}
                        tensor=S.tensor, offset=S.offset,
                        ap=[S.ap[0], [509, 2], [1, 3]],
                    )
                    nc.vector.tensor_copy(
                        V[:, 8:14].rearrange("q (g e) -> q g e", g=2), edges)
                    nc.vector.tensor_sub(V[:, 0:1], V[:, 7:8], V[:, 13:14])
                    nc.vector.memset(V[:, 1:2], 0.0)
                    nc.vector.tensor_copy(V[:, 2:4], V[:, 8:10])
                    nc.vector.tensor_copy(V[:, 4:7], S[:, 508:511])
                    nc.vector.tensor_sub(V[:, 14:15], V[:, 7:8], V[:, 8:9])
                    nc.vector.tensor_copy(V[:, 15:18], S[:, 1:4])
                    nc.vector.tensor_copy(V[:, 18:20], V[:, 12:14])
                    nc.vector.memset(V[:, 20:21], 0.0)
                    ctx["V"] = V

                def stage_r(b, ctx):
                    # ---- phase 2b: Et = R^T [7, 14]; all (q, ci) stats
                    # contract in one matmul per w-shift ----
                    V = ctx["V"]
                    Rt = pr.tile([7, 14], f32, tag="Rt")
                    nc.tensor.matmul(Rt, c_bpat, c_bias, start=True, stop=False)
                    for dw in range(3):
                        nc.tensor.matmul(
                            Rt, V[:, 7 * dw:7 * dw + 7], c_blk[:, dw, :],
                            start=False, stop=(dw == 2))
                    Et = etp.tile([7, 14], f32r, tag="Et")
                    nc.vector.tensor_copy(Et, Rt)
                    ctx["Et"] = Et

                def stage_out(b, ctx):
                    # ---- phase 4: tg = Et^T(co block) @ (L @ A^T); then
                    # out rows via A against tg; one fused store ----
                    Et = ctx["Et"]
                    tg = tgpool.tile([7, CO, 512], f32r, tag="tg")
                    for co in range(CO):
                        tg_ps = ptg.tile([7, 512], f32, tag="tg_ps")
                        nc.tensor.matmul(tg_ps, Et[:, 7 * co:7 * co + 7], c_la,
                                         start=True, stop=True)
                        nc.vector.tensor_copy(tg[:, co, :], tg_ps)
                    ob = obuf.tile([128, CO, 4, 512], f32, tag="ob")
                    for co in range(CO):
                        for t in range(4):
                            oc_ps = poc.tile([128, 512], f32, tag="oc")
                            nc.tensor.matmul(oc_ps, c_atr[:, t, :], tg[:, co, :],
                                             start=True, stop=True)
                            nc.vector.tensor_copy(ob[:, co, t, :], oc_ps)
                        if STORE_SPLIT == 2:
                            nc.gpsimd.dma_start(
                                out=y[b, co].rearrange("(p t) w -> p t w", t=4),
                                in_=ob[:, co],
                            )
                    if STORE_SPLIT == 1:
                        nc.gpsimd.dma_start(
                            out=y[b].rearrange("co (p t) w -> p co t w", t=4),
                            in_=ob,
                        )

                for b in range(BLOC):
                    xt, S = load(b)
                    colsum(b, xt, S)
                    ctx = {}
                    stage_v(b, S, ctx)
                    stage_r(b, ctx)
                    stage_out(b, ctx)

            for _ in range(N_REPS):
                emit_once()
    return nc


def _get_prog():
    global _PROG
    if _PROG is None:
        _PROG = build_program()
        _PROG.finalize()
    return _PROG


# ---------------------------------------------------------------------------
# host entry point
# ---------------------------------------------------------------------------

def _per_core_inputs(x, consts):
    in_maps = []
    for c in range(NCORES):
        xc = np.ascontiguousarray(x[c * BLOC:(c + 1) * BLOC])
        xe = np.empty((BLOC, 2, 4, CIN, W), dtype=np.float32)
        xe[:, 0] = xc[:, :, 0:4, :].transpose(0, 2, 1, 3)
        xe[:, 1] = xc[:, :, H - 4:H, :].transpose(0, 2, 1, 3)
        in_maps.append({
            "xs": xc.reshape(BLOC, 128, 16, W),
            "xe": xe,
            **consts,
        })
    return in_maps


def kernel(x, g_w, g_b, theta_w, theta_b):
    global LAST_EXEC_NS, LAST_TRACE_PATH
    from concourse.bass_utils import run_bass_kernel_spmd

    x = np.ascontiguousarray(np.asarray(x, dtype=np.float32))
    g_w = np.asarray(g_w, dtype=np.float32)
    g_b = np.asarray(g_b, dtype=np.float32)
    theta_w = np.asarray(theta_w, dtype=np.float32)
    theta_b = np.asarray(theta_b, dtype=np.float32)

    consts = build_consts(g_w, g_b, theta_w, theta_b)
    nc = _get_prog()
    in_maps = _per_core_inputs(x, consts)
    try:
        res = run_bass_kernel_spmd(nc, in_maps, core_ids=list(range(NCORES)),
                                   trace=TRACE)
    except ModuleNotFoundError:
        # no NTFF profiling hook in this environment; run untraced
        res = run_bass_kernel_spmd(nc, in_maps, core_ids=list(range(NCORES)),
                                   trace=False)
    LAST_EXEC_NS = res.exec_time_ns
    if TRACE and res.instructions_and_trace is not None:
        LAST_TRACE_PATH = res.instructions_and_trace[1]
    return np.concatenate([res.results[c]["y"] for c in range(NCORES)], axis=0)



# revision 3
# speedup vs baseline: 1.0176x; 1.0176x over previous
"""Trainium2 Bass kernel for nn_CE_55937654063537.

Reference computation:
    b1 = conv3x3(x, g_w) + g_b            [B, 2, 512, 512]
    b2 = conv1x1(x, theta_w) + theta_b    [B, 2, 512, 512]
    m  = patch_mean(b1, 7) + patch_mean(b2, 7)   [B, 2, 7, 7]
    out = bilinear_upsample(m, 512, 512)  (half-pixel centers)

Everything is linear, so the kernel never materializes the conv outputs.
patch_mean(feat)[i, j] is (1/(H*W)) * the sum of feat over a rectangle that is
the full map minus <=3 boundary rows/cols.  Those rectangle sums are linear in
(a) the column-sum over h of x and (b) 8 boundary rows of x.

v3 layout: host reshapes each batch's x [4, 512, 512] to [128, 16, 512]
(partition p = (c, hblock), 32 KB contiguous HBM per partition).  Per batch:
  load:    four 1 MB chunk DMAs alternating the two HWDGE rings; chunked so
           the colsum matmuls start as soon as the first MB lands and the PE
           never idles long enough for HAM to re-throttle
  phase 1: per-channel column sums via an accumulating PE matmul chain
           against a [128, 4] channel-indicator lhsT (f32r, exact for 0/1)
  phase 2: stats reduced to per-row [total + 8 edge columns] (DVE), then tiny
           fp32 matmuls -> R[co, i] summaries
  phase 3: small matmul against L@A^T -> tg[7, co, 512]
  phase 4: out rows via A against tg in four 128-row quadrants per channel;
           PSUM->SBUF copies split across DVE and ACT; 512 KB stores issued
           per (channel, half) on the SWDGE ring as soon as they are ready
All constants arrive in two packed tensors (one DMA each) so the HWDGE rings
carry nothing but x chunks.

Data parallel over batch: 8 cores x 4 batches each; params replicated.
"""
import numpy as np

H = W = 512
K = 7
CIN = 4
CO = 2
BLOC = 4    # batches per core
NCORES = 8

_PROG = None          # cached Bass program (weight-independent; weights are inputs)
N_REPS = 1
NCHUNK = 4            # load DMAs per 4 MB batch
XBUFS = 8             # chunk-tile buffering depth (1 MB each)
TRACE = False
LAST_EXEC_NS = None
LAST_TRACE_PATH = None

# packed f32r const tensor column layout: [128, CRW]
#   0:4      ind        [128, 4]
#   4:516    la = L@A^T [7, 512]
#   516:1028 atrT = A^T [7, 512]   (lhsT for quadrant t is cols 516+128t..)
CR_IND = 0
CR_LA = 4
CR_ATR = 516
CRW = 1028
# packed fp32 const tensor column layout: [36, CFW]
#   0:42     blk        [36, 3*14]
#   42:56    biasrow    [1, 14]
#   56:63    biaspat    [1, 7]
CF_BLK = 0
CF_BIAS = 42
CF_BPAT = 56
CFW = 63


# ---------------------------------------------------------------------------
# host-side constant builders (all tiny, derived from conv weights)
# ---------------------------------------------------------------------------

def resize_mat(in_size, out_size):
    """Bilinear (half-pixel, edge-normalized) interpolation matrix [out, in],
    matching jax.image.resize(method='bilinear') for upsampling."""
    inv_scale = in_size / out_size
    sample_f = (np.arange(out_size) + 0.5) * inv_scale - 0.5
    xw = np.abs(sample_f[None, :] - np.arange(in_size)[:, None])
    weights = np.maximum(0, 1 - xw)
    total = weights.sum(axis=0, keepdims=True)
    return (weights / total).T.astype(np.float32)  # [out, in]


def build_lhsTR(g_w, g_b, theta_w, theta_b):
    """Phase-2 weight blocks (per batch; identical for every b).

    Returns (blk [4, 3, 9, 14], bias [1, 14]):
      blk[ci, dw, q, col]: coefficient of stats row q of channel ci
        (q: 0=colsum over h, 1..4=x rows 0..3, 5..8=x rows 508..511)
        in output row col = co*7 + i -> R[co, i][w] under w-shift dw.
      bias[0, col]: additive constant (applies to every w of R[col]).
    """
    gw = g_w.astype(np.float64)
    gb = g_b.astype(np.float64)
    tw = theta_w.astype(np.float64)[:, :, 0, 0]
    tb = theta_b.astype(np.float64)
    blk = np.zeros((CIN, 3, 9, 14), dtype=np.float64)
    bias = np.zeros((1, 14), dtype=np.float64)

    def add_F(col, co, dw, sign):
        for ci in range(CIN):
            blk[ci, dw, 0, col] += sign * gw[co, ci, :, dw].sum()
            blk[ci, dw, 1, col] += -sign * gw[co, ci, 2, dw]   # x row 0
            blk[ci, dw, 8, col] += -sign * gw[co, ci, 0, dw]   # x row 511
            if dw == 1:
                blk[ci, dw, 0, col] += sign * tw[co, ci]
        if dw == 1:
            bias[0, col] += sign * H * (gb[co] + tb[co])

    def add_bd(col, co, r, dw, sign):
        for ci in range(CIN):
            for dh in range(3):
                hr = r + dh - 1
                if 0 <= hr < H:
                    q = 1 + hr if hr <= 3 else 5 + (hr - (H - 4))
                    blk[ci, dw, q, col] += sign * gw[co, ci, dh, dw]
            if dw == 1:
                q = 1 + r if r <= 3 else 5 + (r - (H - 4))
                blk[ci, dw, q, col] += sign * tw[co, ci]
        if dw == 1:
            bias[0, col] += sign * (gb[co] + tb[co])

    for co in range(CO):
        for i in range(K):
            col = co * 7 + i
            for dw in range(3):
                add_F(col, co, dw, 1.0)
                if i < 3:
                    for r in range(H - 3 + i, H):
                        add_bd(col, co, r, dw, -1.0)
                elif i > 3:
                    for r in range(0, i - 3):
                        add_bd(col, co, r, dw, -1.0)
    return blk.astype(np.float32), bias.astype(np.float32)


def build_L():
    """Phase-3 lhsT [7, 7] (includes the 1/(H*W) patch-mean scale)."""
    L = np.zeros((7, 7), dtype=np.float64)
    L[0, :] = 1.0
    for j in range(3):            # j=0,1,2: subtract tail elements w >= 509+j
        for e in range(3 + j, 6):
            L[1 + e, j] = -1.0    # e=3,4,5 -> rows 4..6
    for j in range(4, 7):         # j=4,5,6: subtract head elements w < j-3
        for e in range(0, j - 3):
            L[1 + e, j] = -1.0    # e=0,1,2 -> rows 1..3
    return (L / (H * W)).astype(np.float32)


def build_consts(g_w, g_b, theta_w, theta_b):
    blk, biasrow = build_lhsTR(g_w, g_b, theta_w, theta_b)
    A = resize_mat(K, H)          # [512, 7]
    biaspat = np.ones((1, 7), dtype=np.float32)
    biaspat[0, 0] = float(W)      # total-sum column gets bias once per w
    la = build_L().astype(np.float64) @ A.astype(np.float64).T     # [7, 512]
    # stats live on 36 partitions: q*4 + ci (q: 0=colsum, 1..8=edge rows)
    blk4 = np.ascontiguousarray(blk.transpose(2, 0, 1, 3).reshape(36, 3, 14))

    cr = np.zeros((128, CRW), dtype=np.float32)
    for c in range(CIN):          # channel indicator
        cr[32 * c:32 * (c + 1), CR_IND + c] = 1.0
    cr[0:7, CR_LA:CR_LA + 512] = la.astype(np.float32)
    cr[0:7, CR_ATR:CR_ATR + 512] = A.T                             # [7, 512]

    cf = np.zeros((36, CFW), dtype=np.float32)
    cf[:, CF_BLK:CF_BLK + 42] = blk4.reshape(36, 42)
    cf[0:1, CF_BIAS:CF_BIAS + 14] = biasrow
    cf[0:1, CF_BPAT:CF_BPAT + 7] = biaspat
    return {"cr": cr, "cf": cf}


# ---------------------------------------------------------------------------
# device program
# ---------------------------------------------------------------------------

def build_program():
    import concourse.bass as bass
    import concourse.bacc as bacc
    import concourse.tile as tile
    from concourse import mybir

    f32 = mybir.dt.float32
    f32r = mybir.dt.float32r
    nc = bacc.Bacc(None, target_bir_lowering=False, enable_partition_id=False)

    xs = nc.dram_tensor("xs", [BLOC, 128, 16, W], f32r, kind="ExternalInput")
    xe_d = nc.dram_tensor("xe", [BLOC, 2, 4, CIN, W], f32r, kind="ExternalInput")
    cr_d = nc.dram_tensor("cr", [128, CRW], f32r, kind="ExternalInput")
    cf_d = nc.dram_tensor("cf", [36, CFW], f32, kind="ExternalInput")
    y = nc.dram_tensor("y", [BLOC, CO, H, W], f32, kind="ExternalOutput")

    TPC = 16 // NCHUNK            # t-tiles per load chunk

    with tile.TileContext(nc) as tc:
        with (
            tc.tile_pool(name="consts", bufs=1) as consts,
            tc.tile_pool(name="xpool", bufs=XBUFS) as xpool,
            tc.tile_pool(name="spool", bufs=5) as spool,
            tc.tile_pool(name="vpool", bufs=2) as vpool,
            tc.tile_pool(name="etp", bufs=2) as etp,
            tc.tile_pool(name="tgpool", bufs=2) as tgpool,
            tc.tile_pool(name="opool", bufs=4) as opool,
            tc.tile_pool(name="pstats", bufs=2, space="PSUM") as pstats,
            tc.tile_pool(name="pr", bufs=1, space="PSUM") as pr,
            tc.tile_pool(name="ptg", bufs=2, space="PSUM") as ptg,
            tc.tile_pool(name="poc", bufs=3, space="PSUM") as poc,
        ):
            # constants + per-batch edge rows ride the SWDGE queue up front;
            # the HWDGE rings carry nothing but x chunks until stores begin
            c_r = consts.tile([128, CRW], f32r)
            nc.gpsimd.dma_start(out=c_r, in_=cr_d[:, :])
            c_f = consts.tile([36, CFW], f32)
            nc.gpsimd.dma_start(out=c_f, in_=cf_d[:, :])
            c_ind = c_r[:, CR_IND:CR_IND + CIN]
            c_la = c_r[0:7, CR_LA:CR_LA + 512]
            c_bias = c_f[0:1, CF_BIAS:CF_BIAS + 14]
            c_bpat = c_f[0:1, CF_BPAT:CF_BPAT + 7]

            def emit_once():
                Ss = []
                for b in range(BLOC):
                    S = spool.tile([36, W], f32r, tag="S")
                    nc.gpsimd.dma_start(
                        out=S[4:36, :],
                        in_=xe_d[b].rearrange("e r c w -> (e r c) w"),
                    )
                    Ss.append(S)

                def load_colsum(b):
                    # four 1 MB chunks, alternating HWDGE rings; 4 colsum
                    # matmuls per chunk keep the PE dense enough to stay warm
                    st = pstats.tile([CIN, W], f32, tag="st")
                    for k in range(NCHUNK):
                        xt = xpool.tile([128, TPC, W], f32r, tag="xt")
                        eng = nc.sync if (b * NCHUNK + k) % 2 == 0 else nc.scalar
                        eng.dma_start(
                            out=xt, in_=xs[b, :, k * TPC:(k + 1) * TPC, :])
                        for j in range(TPC):
                            nc.tensor.matmul(
                                st, c_ind, xt[:, j, :],
                                start=(k == 0 and j == 0),
                                stop=(k == NCHUNK - 1 and j == TPC - 1))
                    return st

                def stage_v(b, st):
                    # stats -> V = [T | edge columns], three 7-col groups,
                    # one per w-shift dw of the 3x3 conv
                    S = Ss[b]
                    nc.vector.tensor_copy(S[0:CIN, :], st)
                    V = vpool.tile([36, 21], f32, tag="V")
                    nc.vector.reduce_sum(V[:, 7:8], S, axis=mybir.AxisListType.X)
                    edges = bass.AP(           # S columns {0,1,2, 509,510,511}
                        tensor=S.tensor, offset=S.offset,
                        ap=[S.ap[0], [509, 2], [1, 3]],
                    )
                    nc.vector.tensor_copy(
                        V[:, 8:14].rearrange("q (g e) -> q g e", g=2), edges)
                    nc.vector.tensor_sub(V[:, 0:1], V[:, 7:8], V[:, 13:14])
                    nc.vector.memset(V[:, 1:2], 0.0)
                    nc.vector.tensor_copy(V[:, 2:4], V[:, 8:10])
                    nc.vector.tensor_copy(V[:, 4:7], S[:, 508:511])
                    nc.vector.tensor_sub(V[:, 14:15], V[:, 7:8], V[:, 8:9])
                    nc.vector.tensor_copy(V[:, 15:18], S[:, 1:4])
                    nc.vector.tensor_copy(V[:, 18:20], V[:, 12:14])
                    nc.vector.memset(V[:, 20:21], 0.0)
                    return V

                def stage_r(b, V):
                    # Et = R^T [7, 14]; all (q, ci) stats contract in one
                    # matmul per w-shift
                    Rt = pr.tile([7, 14], f32, tag="Rt")
                    nc.tensor.matmul(Rt, c_bpat, c_bias, start=True, stop=False)
                    for dw in range(3):
                        nc.tensor.matmul(
                            Rt, V[:, 7 * dw:7 * dw + 7],
                            c_f[:, CF_BLK + 14 * dw:CF_BLK + 14 * dw + 14],
                            start=False, stop=(dw == 2))
                    Et = etp.tile([7, 14], f32r, tag="Et")
                    nc.vector.tensor_copy(Et, Rt)
                    return Et

                def stage_out(b, Et):
                    # tg = Et^T(co block) @ (L @ A^T); then out rows via A^T
                    # against tg in 128-row quadrants; copies split DVE/ACT;
                    # one 512 KB store per (co, half)
                    tg = tgpool.tile([7, CO, 512], f32r, tag="tg")
                    for co in range(CO):
                        tg_ps = ptg.tile([7, 512], f32, tag="tg_ps")
                        nc.tensor.matmul(tg_ps, Et[:, 7 * co:7 * co + 7], c_la,
                                         start=True, stop=True)
                        nc.scalar.copy(tg[:, co, :], tg_ps)
                    for co in range(CO):
                        for g in range(2):
                            ot = opool.tile([128, 2, 512], f32, tag="ot")
                            for tt in range(2):
                                t = 2 * g + tt
                                oc_ps = poc.tile([128, 512], f32, tag="oc")
                                nc.tensor.matmul(
                                    oc_ps,
                                    c_r[0:7, CR_ATR + 128 * t:CR_ATR + 128 * (t + 1)],
                                    tg[:, co, :], start=True, stop=True)
                                if (co + tt) % 2 == 0:
                                    nc.vector.tensor_copy(ot[:, tt, :], oc_ps)
                                else:
                                    nc.scalar.copy(ot[:, tt, :], oc_ps)
                            nc.gpsimd.dma_start(
                                out=y[b, co, 256 * g:256 * (g + 1), :]
                                    .rearrange("(tt p) w -> p tt w", tt=2),
                                in_=ot,
                            )

                for b in range(BLOC):
                    st = load_colsum(b)
                    V = stage_v(b, st)
                    Et = stage_r(b, V)
                    stage_out(b, Et)

            for _ in range(N_REPS):
                emit_once()
    return nc


def _get_prog():
    global _PROG
    if _PROG is None:
        _PROG = build_program()
        _PROG.finalize()
    return _PROG


# ---------------------------------------------------------------------------
# host entry point
# ---------------------------------------------------------------------------

def _per_core_inputs(x, consts):
    in_maps = []
    for c in range(NCORES):
        xc = np.ascontiguousarray(x[c * BLOC:(c + 1) * BLOC])
        xe = np.empty((BLOC, 2, 4, CIN, W), dtype=np.float32)
        xe[:, 0] = xc[:, :, 0:4, :].transpose(0, 2, 1, 3)
        xe[:, 1] = xc[:, :, H - 4:H, :].transpose(0, 2, 1, 3)
        in_maps.append({
            "xs": xc.reshape(BLOC, 128, 16, W),
            "xe": xe,
            **consts,
        })
    return in_maps


def kernel(x, g_w, g_b, theta_w, theta_b):
    global LAST_EXEC_NS, LAST_TRACE_PATH
    from concourse.bass_utils import run_bass_kernel_spmd

    x = np.ascontiguousarray(np.asarray(x, dtype=np.float32))
    g_w = np.asarray(g_w, dtype=np.float32)
    g_b = np.asarray(g_b, dtype=np.float32)
    theta_w = np.asarray(theta_w, dtype=np.float32)
    theta_b = np.asarray(theta_b, dtype=np.float32)

    consts = build_consts(g_w, g_b, theta_w, theta_b)
    nc = _get_prog()
    in_maps = _per_core_inputs(x, consts)
    try:
        res = run_bass_kernel_spmd(nc, in_maps, core_ids=list(range(NCORES)),
                                   trace=TRACE)
    except ModuleNotFoundError:
        # no NTFF profiling hook in this environment; run untraced
        res = run_bass_kernel_spmd(nc, in_maps, core_ids=list(range(NCORES)),
                                   trace=False)
    LAST_EXEC_NS = res.exec_time_ns
    if TRACE and res.instructions_and_trace is not None:
        LAST_TRACE_PATH = res.instructions_and_trace[1]
    return np.concatenate([res.results[c]["y"] for c in range(NCORES)], axis=0)
